# revision 10
# baseline (speedup 1.0000x reference)
"""AdaConv Trainium2 kernel: instance-norm + per-sample (3x3 conv @ 1x1 conv)
+ bias + shared 3x3 conv, NHWC, reflect padding.

Strategy: pure data parallel over batch (16 samples -> 8 cores x 2).
Host side only re-lays-out data (transpose to channels-major, bf16 cast);
all model math (stats, normalize, kernel-combine, convs) runs on-device.

Per-core layouts (all channels-major so conv = 9 shifted matmuls):
  xt    [2, 256, 64, 64]  bf16   x transposed
  dkt   [2, 256, 9, 256]  bf16   depthwise_kernels as [cm, tap, ci]
  pk    [2, 256, 256]     bf16   pointwise_kernels as [cm, co]
  bias  [2, 256, 1]       f32
  convw [256, 9, 256]     bf16   conv_w as [ci, tap, co]  (replicated)
  convb [256, 1]          f32    (replicated)
  out   [2, 256, 8, 8, 64] f32   y as [co, sp_tile, row, col]

Math notes:
 - per-sample 3x3 (dk) followed by 1x1 (pk) == single 3x3 with
   comb[tap,ci,co] = sum_cm dk[tap,ci,cm] * pk[cm,co]; computed on PE.
 - conv as matmul: psum[co, pix] += comb[tap][ci,co].T @ xpad[ci, pix+tap],
   accumulated over 9 taps x 2 ci chunks in PSUM.
 - padded activations stored as [128, 66, 68] (68-wide rows keep the
   64-wide interior writes 4-byte aligned for DVE/ACT fast modes).
   Padded image col j lives at sbuf col 1+j; cols 0 and 67 are unused.
"""

import numpy as np
import ml_dtypes

from concourse import bacc, mybir
import concourse.tile as tile
from concourse.bass_utils import run_bass_kernel_spmd

BF16 = mybir.dt.bfloat16
F32 = mybir.dt.float32
ALU = mybir.AluOpType
ACT = mybir.ActivationFunctionType

N_CORES = 8
SPB = 2  # samples per core
EPS = 1e-3
INV_N = 1.0 / 4096.0

_cache = {}


def _pad_edges(nc, xp):
    """Reflect-pad edge fixups for a [128, 66, 68] tile whose interior
    (padded rows 1..64, padded cols 1..64 -> sbuf cols 2..65) is written.
    Padded col j <-> sbuf col 1+j."""
    v = nc.vector
    # left pad (j=0 <- j=2): sbuf col 1 <- col 3 ; right pad (j=65 <- j=63)
    v.tensor_copy(out=xp[:, 1:65, 1:2], in_=xp[:, 1:65, 3:4])
    v.tensor_copy(out=xp[:, 1:65, 66:67], in_=xp[:, 1:65, 64:65])
    # top/bottom pad rows, full padded width (sbuf cols 1..66)
    v.tensor_copy(out=xp[:, 0:1, 1:67], in_=xp[:, 2:3, 1:67])
    v.tensor_copy(out=xp[:, 65:66, 1:67], in_=xp[:, 63:64, 1:67])


def _build():
    nc = bacc.Bacc("TRN2", target_bir_lowering=False, debug=False,
                   num_devices=N_CORES)

    xt = nc.dram_tensor("xt", [SPB, 256, 64, 64], BF16, kind="ExternalInput").ap()
    dkt = nc.dram_tensor("dkt", [SPB, 256, 9, 256], BF16, kind="ExternalInput").ap()
    pk = nc.dram_tensor("pk", [SPB, 256, 256], BF16, kind="ExternalInput").ap()
    bias = nc.dram_tensor("bias", [SPB, 256, 1], F32, kind="ExternalInput").ap()
    convw = nc.dram_tensor("convw", [256, 9, 256], BF16, kind="ExternalInput").ap()
    convb = nc.dram_tensor("convb", [256, 1], F32, kind="ExternalInput").ap()
    out = nc.dram_tensor("out", [SPB, 256, 8, 8, 64], F32, kind="ExternalOutput").ap()

    with tile.TileContext(nc) as tc:
        with (
            tc.tile_pool(name="main", bufs=2) as pool,
            tc.tile_pool(name="psconv", bufs=4, space="PSUM") as psconv,
            tc.tile_pool(name="pscomb", bufs=2, space="PSUM") as pscomb,
        ):
            # PE warmup: ~4.5us of dummy matmuls so the HAM clock-gate is
            # released before the first real matmuls arrive.
            wsrc = pool.tile([128, 512], BF16, tag="wsrc", bufs=1)
            nc.gpsimd.memset(wsrc, 0.0)
            for k in range(17):
                wp = pscomb.tile([128, 512], F32, tag="kps", name="warmps")
                nc.tensor.matmul(wp, lhsT=wsrc[:, :128], rhs=wsrc,
                                 start=True, stop=True)

            # shared conv weights (needed only by conv3, ~100us in) on the
            # SWDGE queue so they don't delay the per-sample loads.
            convw_sb, convb_sb = [], []
            for c in range(2):
                w = pool.tile([128, 9, 256], BF16, tag=f"convw{c}", bufs=1)
                nc.gpsimd.dma_start(out=w, in_=convw[c * 128:(c + 1) * 128])
                convw_sb.append(w)
                b = pool.tile([128, 1], F32, tag=f"convb{c}", bufs=1)
                nc.gpsimd.dma_start(out=b, in_=convb[c * 128:(c + 1) * 128])
                convb_sb.append(b)
            eps_sb = pool.tile([128, 1], F32, tag="eps", bufs=1)
            nc.vector.memset(eps_sb, EPS)
            # preload the SQRT activation table off the critical path
            sqrt_warm = pool.tile([128, 1], F32, tag="sqrt_warm", bufs=1)
            nc.scalar.activation(out=sqrt_warm, in_=eps_sb, func=ACT.Sqrt,
                                 bias=eps_sb, scale=1.0)

            for s in range(SPB):
                # ---- loads, spread across the three DMA queues ------------
                x_t, dkt_sb, pk_sb, bias_sb = [], [], [], []
                for c in range(2):
                    cs = slice(c * 128, (c + 1) * 128)
                    t = pool.tile([128, 64, 64], BF16, tag=f"xt{c}", name=f"xt_sb{c}")
                    (nc.scalar if c == 0 else nc.gpsimd).dma_start(
                        out=t, in_=xt[s, cs])
                    x_t.append(t)
                for c in range(2):
                    cs = slice(c * 128, (c + 1) * 128)
                    t = pool.tile([128, 9, 256], BF16, tag=f"dkt{c}")
                    nc.sync.dma_start(out=t, in_=dkt[s, cs])
                    dkt_sb.append(t)
                    t = pool.tile([128, 256], BF16, tag=f"pk{c}")
                    nc.scalar.dma_start(out=t, in_=pk[s, cs])
                    pk_sb.append(t)
                    t = pool.tile([128, 1], F32, tag=f"bias{c}")
                    nc.sync.dma_start(out=t, in_=bias[s, cs])
                    bias_sb.append(t)

                # ---- instance-norm stats ----------------------------------
                # sum(x) on DVE, sum(x^2) on ACT, in parallel. Both chunks'
                # big passes are emitted before any chain op so the ACT FIFO
                # doesn't head-of-line block Square(c1) behind Sqrt(c0).
                sums, sqs = [], []
                for c in range(2):
                    scr1 = pool.tile([128, 64, 64], BF16, tag=f"scr1_{c}", bufs=1)
                    scr2 = pool.tile([128, 64, 64], BF16, tag=f"scr2_{c}", bufs=1)
                    sum_ = pool.tile([128, 1], F32, tag=f"sum{c}")
                    sq_ = pool.tile([128, 1], F32, tag=f"sq{c}")
                    nc.vector.scalar_tensor_tensor(
                        out=scr1, in0=x_t[c], scalar=0.0, in1=x_t[c],
                        op0=ALU.mult, op1=ALU.add, accum_out=sum_)
                    nc.scalar.activation(
                        out=scr2, in_=x_t[c], func=ACT.Square, accum_out=sq_)
                    sums.append(sum_)
                    sqs.append(sq_)

                means, alphas = [], []
                for c in range(2):
                    mean = pool.tile([128, 1], F32, tag=f"mean{c}")
                    ex2 = pool.tile([128, 1], F32, tag=f"ex2{c}")
                    var = pool.tile([128, 1], F32, tag=f"var{c}")
                    std = pool.tile([128, 1], F32, tag=f"std{c}")
                    alpha = pool.tile([128, 1], F32, tag=f"alpha{c}")
                    nc.vector.tensor_scalar_mul(out=mean, in0=sums[c], scalar1=INV_N)
                    nc.vector.tensor_scalar_mul(out=ex2, in0=sqs[c], scalar1=INV_N)
                    nc.vector.tensor_tensor(out=var, in0=mean, in1=mean, op=ALU.mult)
                    nc.vector.tensor_tensor(out=var, in0=ex2, in1=var, op=ALU.subtract)
                    nc.scalar.activation(out=std, in_=var, func=ACT.Sqrt,
                                         bias=eps_sb, scale=1.0)
                    nc.vector.reciprocal(out=alpha, in_=std)
                    means.append(mean)
                    alphas.append(alpha)

                xp1 = []
                for c in range(2):
                    xp = pool.tile([128, 66, 68], BF16, tag=f"xp1_{c}")
                    nc.vector.tensor_scalar_sub(
                        out=xp[:, 1:65, 2:66], in0=x_t[c], scalar1=means[c])
                    _pad_edges(nc, xp)
                    xp1.append(xp)

                # ---- combine per-sample kernels: comb = alpha * (dk @ pk) -
                comb = []
                for cic in range(2):
                    cb = pool.tile([128, 9, 256], BF16, tag=f"comb{cic}")
                    for tg in range(3):
                        kps = pscomb.tile([128, 3, 256], F32, tag="kps")
                        for ti in range(3):
                            tap = tg * 3 + ti
                            for cmc in range(2):
                                nc.tensor.matmul(
                                    kps[:, ti, :],
                                    lhsT=dkt_sb[cmc][:, tap, cic * 128:(cic + 1) * 128],
                                    rhs=pk_sb[cmc],
                                    start=(cmc == 0), stop=(cmc == 1))
                        nc.vector.tensor_scalar_mul(
                            out=cb[:, tg * 3:(tg + 1) * 3, :], in0=kps,
                            scalar1=alphas[cic])
                    comb.append(cb)

                # ---- conv1 (combined per-sample 3x3) + bias -> xpad2 ------
                xp2 = [pool.tile([128, 66, 68], BF16, tag=f"xp2_{c}",
                                 name=f"xp2_{c}") for c in range(2)]
                for coc in range(2):
                    for sp in range(8):
                        ps = psconv.tile([128, 8, 64], F32, tag="cps")
                        i = 0
                        for cic in range(2):
                            for tap in range(9):
                                ky, kx = divmod(tap, 3)
                                nc.tensor.matmul(
                                    ps,
                                    lhsT=comb[cic][:, tap, coc * 128:(coc + 1) * 128],
                                    rhs=xp1[cic][:, 8 * sp + ky:8 * sp + ky + 8,
                                                 1 + kx:65 + kx],
                                    start=(i == 0), stop=(i == 17))
                                i += 1
                        nc.scalar.activation(
                            out=xp2[coc][:, 1 + 8 * sp:9 + 8 * sp, 2:66],
                            in_=ps, func=ACT.Identity, bias=bias_sb[coc], scale=1.0)
                for c in range(2):
                    _pad_edges(nc, xp2[c])

                # ---- conv3 (shared 3x3) + conv_b -> out -------------------
                for coc in range(2):
                    for sp in range(8):
                        ps = psconv.tile([128, 8, 64], F32, tag="cps")
                        i = 0
                        for cic in range(2):
                            for tap in range(9):
                                ky, kx = divmod(tap, 3)
                                nc.tensor.matmul(
                                    ps,
                                    lhsT=convw_sb[cic][:, tap, coc * 128:(coc + 1) * 128],
                                    rhs=xp2[cic][:, 8 * sp + ky:8 * sp + ky + 8,
                                                 1 + kx:65 + kx],
                                    start=(i == 0), stop=(i == 17))
                                i += 1
                        osb = pool.tile([128, 8, 64], F32, tag="osb", bufs=4)
                        nc.scalar.activation(out=osb, in_=ps, func=ACT.Identity,
                                             bias=convb_sb[coc], scale=1.0)
                        nc.scalar.dma_start(
                            out=out[s, coc * 128:(coc + 1) * 128, sp], in_=osb)

    nc.compile()
    return nc


def _get_nc():
    if "nc" not in _cache:
        _cache["nc"] = _build()
    return _cache["nc"]


def _prep_in_maps(x, depthwise_kernels, pointwise_kernels, biases, conv_w, conv_b):
    bf = ml_dtypes.bfloat16
    B = x.shape[0]
    x = np.asarray(x, np.float32)
    xt = np.ascontiguousarray(x.transpose(0, 3, 1, 2)).astype(bf)  # [B,256,64,64]
    dk = np.asarray(depthwise_kernels, np.float32)
    dkt = np.ascontiguousarray(dk.transpose(0, 4, 1, 2, 3)).reshape(
        B, 256, 9, 256).astype(bf)  # [B, cm, tap, ci]
    pkm = np.asarray(pointwise_kernels, np.float32).reshape(B, 256, 256).astype(bf)
    bb = np.asarray(biases, np.float32).reshape(B, 256, 1)
    cw = np.ascontiguousarray(
        np.asarray(conv_w, np.float32).transpose(2, 0, 1, 3)).reshape(
        256, 9, 256).astype(bf)  # [ci, tap, co]
    cb = np.asarray(conv_b, np.float32).reshape(256, 1)

    in_maps = []
    for i in range(N_CORES):
        sl = slice(i * SPB, (i + 1) * SPB)
        in_maps.append(dict(
            xt=np.ascontiguousarray(xt[sl]),
            dkt=np.ascontiguousarray(dkt[sl]),
            pk=np.ascontiguousarray(pkm[sl]),
            bias=np.ascontiguousarray(bb[sl]),
            convw=cw, convb=cb))
    return in_maps


def run(inputs, trace=False, tmpdir=None):
    """Returns (full_output [16,64,64,256] f32, exec_time_ns or None)."""
    nc = _get_nc()
    in_maps = _prep_in_maps(**inputs)
    res = run_bass_kernel_spmd(nc, in_maps, list(range(N_CORES)),
                               trace=trace, tmpdir=tmpdir)
    parts = [res.results[i]["out"].reshape(SPB, 256, 4096) for i in range(N_CORES)]
    y = np.concatenate(parts, axis=0).reshape(16, 256, 64, 64)
    y = np.ascontiguousarray(y.transpose(0, 2, 3, 1))
    return y, res.exec_time_ns


def kernel(**inputs):
    y, _ = run(inputs, trace=False)
    return y
